# revision 9
# baseline (speedup 1.0000x reference)
"""Trainium2 Bass kernel for nn_Denoising_ResNet: out = x + conv1x1(box_mean3x3(x)) + b.

Sharding: data-parallel over batch (32 samples -> 4 per core x 8 cores).

Per-core layout: 2 "stacks" of 2 samples each -> 128 SBUF partitions
(= 2 samples x 64 channels). Each stack's full image is cast-loaded to
one bf16 SBUF tile by 4 quarter DMAs whose boundaries (34/66/98) align
with the 32-row compute chunks' halo spans, so chunk ci only waits on
quarters 0..ci.

Math decomposition (K=3 edge-clipped box mean, then 1x1 conv):
  - W-direction 3-tap sum: two shifted DVE adds (bf16, 2x DVE mode);
    edge cols folded to 1.5*(2-tap).
  - Global /9 of the box mean is folded into the conv weight.
  - H-direction 3-tap sum is FUSED into the 1x1 conv: 3 accumulating
    PE matmuls with row-shifted moving operands against a block-diagonal
    [128,128] weight kron(I2, (W/9)^T) in bf16; zeroed ws halo rows give
    the edge-clipped sum at image boundaries.
  - The residual +x rides the same PSUM group as a 4th accumulating
    identity matmul (kron(I2, I) bf16).
  - Image-boundary rows get 2 extra in-group matmuls against 0.5x the
    conv weight (-> 1.5x conv total = the edge-clip row count fix,
    leaving x and bias unscaled).
  - One accumulation group per 2KB PSUM bank (start=True zeroes the
    whole bank); matmuls are emitted grouped by stationary weight.
  - PSUM -> SBUF copy + bias live on the scalar engine
    (activation Identity, bias per partition), one per 2-bank PSUM tile.

Measured: ~95.2us HW exec vs ~94us HBM roofline (16.8MB read +
16.8MB write per core at ~358 GB/s/core).
"""
from contextlib import ExitStack

import bass_rust
import numpy as np

import concourse.bass as bass
import concourse.tile as tile
from concourse import bacc, mybir
from concourse.bass_utils import run_bass_kernel_spmd

B, C, H, W = 32, 64, 128, 128
NCORES = 8
PER = B // NCORES  # samples per core
NSTACK = PER // 2  # 2-sample stacks per core
HC = 32  # chunk height (output rows per chunk)
NCHUNK = H // HC
GROUP_ROWS = 4  # rows per matmul accumulation group (512 f32 = 1 bank)
TILE_ROWS = 8  # rows per PSUM tile (2 banks), 2 groups per tile
NTILE = HC // TILE_ROWS

F32 = mybir.dt.float32
BF16 = mybir.dt.bfloat16


def _rows_ap(t, base_row, dims):
    """AP over a [128, R, W] tile starting at base_row, with custom row dims.

    dims: list of (row_stride_in_rows, size) free dims; an inner (1, W) dim is
    appended. Strides may be 0 (repeat) or overlap — used to stream repeated /
    permuted row patterns into the PE without materializing them in SBUF.
    """
    ap = t[:, base_row : base_row + 1, :].copy()
    part = ap.ap.to_list()[0]
    ap.ap = bass_rust.VecI64Pair([part] + [[s * W, n] for s, n in dims] + [[1, W]])
    return ap


def _build_nc() -> bass.Bass:
    nc = bacc.Bacc("TRN2", debug=False)
    x = nc.dram_tensor("x", [PER * C, H, W], BF16, kind="ExternalInput")
    w9t = nc.dram_tensor("w9t", [2 * C, 2 * C], BF16, kind="ExternalInput")
    ident = nc.dram_tensor("ident", [2 * C, 2 * C], BF16, kind="ExternalInput")
    w05t = nc.dram_tensor("w05t", [2 * C, 2 * C], BF16, kind="ExternalInput")
    bias2 = nc.dram_tensor("bias2", [2 * C, 1], F32, kind="ExternalInput")
    y = nc.dram_tensor("y", [PER * C, H, W], BF16, kind="ExternalOutput")
    xap = x.ap()
    yap = y.ap()
    IDENT_FN = mybir.ActivationFunctionType.Identity

    with ExitStack() as ctx:
        tc = ctx.enter_context(tile.TileContext(nc))
        cpool = ctx.enter_context(tc.tile_pool(name="const", bufs=1))
        wt = cpool.tile([128, 128], BF16)
        nc.sync.dma_start(out=wt[:], in_=w9t.ap()[:, :])
        it = cpool.tile([128, 128], BF16)
        nc.sync.dma_start(out=it[:], in_=ident.ap()[:, :])
        w05 = cpool.tile([128, 128], BF16)
        nc.sync.dma_start(out=w05[:], in_=w05t.ap()[:, :])
        bt = cpool.tile([128, 1], F32)
        nc.sync.dma_start(out=bt[:], in_=bias2.ap()[:, :])

        ppool = ctx.enter_context(tc.tile_pool(name="psum", bufs=4, space="PSUM"))

        xpool = ctx.enter_context(tc.tile_pool(name="xin", bufs=2))
        tpool = ctx.enter_context(tc.tile_pool(name="tmp", bufs=2))
        wpool = ctx.enter_context(tc.tile_pool(name="wsum", bufs=2))
        qpool = ctx.enter_context(tc.tile_pool(name="qsum", bufs=2))
        opool = ctx.enter_context(tc.tile_pool(name="out", bufs=4))

        for g in range(NSTACK):
            p0 = g * 128
            # whole-stack bf16 image tile, filled by 4 quarter cast-DMAs
            # (no halo re-reads; chunks slice it with their halos).
            xt = xpool.tile([128, H, W], BF16)
            # quarter boundaries at 34/66/98: chunk ci's halo span
            # [32*ci-1, 32*ci+33) is covered by quarters 0..ci, so each
            # chunk waits only one new quarter-DMA.
            qb = [0, HC + 2, 2 * HC + 2, 3 * HC + 2, H]
            for q in range(4):
                nc.gpsimd.dma_start(
                    out=xt[:, qb[q] : qb[q + 1], :],
                    in_=xap[p0 : p0 + 128, qb[q] : qb[q + 1], :],
                )
            for ci in range(NCHUNK):
                h0 = ci * HC
                # chunk rows [h0, h0+HC); halo rows clamped at the image edge
                ra = 0 if ci == 0 else h0 - 1       # first xt row read
                rb = h0 + HC if ci == NCHUNK - 1 else h0 + HC + 1

                # W-direction 3-tap on DVE (bf16). tt/ws row r holds
                # image row h0-1+r; out-of-image halo ws rows are zeroed.
                la = ra - (h0 - 1)  # first valid local row (0 or 1)
                lb = rb - (h0 - 1)  # past-last valid local row
                tt = tpool.tile([128, HC + 2, W], BF16)
                ws = wpool.tile([128, HC + 2, W], BF16)
                if ci == 0:
                    nc.vector.memset(ws[:, 0:1, :], 0.0)
                elif ci == NCHUNK - 1:
                    nc.vector.memset(ws[:, HC + 1 : HC + 2, :], 0.0)
                nc.vector.tensor_add(
                    tt[:, la:lb, 1:W], xt[:, ra:rb, 0 : W - 1], xt[:, ra:rb, 1:W]
                )
                nc.vector.tensor_add(
                    ws[:, la:lb, 1 : W - 1], tt[:, la:lb, 1 : W - 1], xt[:, ra:rb, 2:W]
                )
                nc.vector.tensor_scalar_mul(ws[:, la:lb, 0:1], tt[:, la:lb, 1:2], 1.5)
                nc.vector.tensor_scalar_mul(
                    ws[:, la:lb, W - 1 : W], tt[:, la:lb, W - 1 : W], 1.5
                )

                # even-aligned H pair sums on the (otherwise idle) Pool engine:
                # q[j] = ws_img[h0+2j] + ws_img[h0+2j+1]  (locals 2j+1, 2j+2)
                qt = qpool.tile([128, HC // 2, W], BF16)
                nc.gpsimd.tensor_add(
                    qt[:], ws[:, 1 : HC + 1 : 2, :], ws[:, 2 : HC + 2 : 2, :]
                )

                ot = opool.tile([128, HC, W], BF16)
                for tp in range(NTILE):
                    ps = ppool.tile([128, TILE_ROWS, W], F32, tag="ps")
                    t0 = tp * TILE_ROWS  # chunk-local first output row of tile
                    # H-direction 3-tap fused into the conv with only 2 PE
                    # passes per 4-row group via the pair sums:
                    #   even r: hsum[r] = q[r/2]     + ws[r-1]
                    #   odd  r: hsum[r] = q[(r-1)/2] + ws[r+1]
                    # pass A streams q rows (j0,j0,j0+1,j0+1) [stride-0 repeat],
                    # pass B streams ws locals l0+(0,3,2,5) [overlapping dims].
                    for hp in range(2):
                        ga, gb = hp * GROUP_ROWS, (hp + 1) * GROUP_ROWS
                        l0 = t0 + ga  # chunk-local first output row of group
                        apA = _rows_ap(qt, l0 // 2, [(1, 2), (0, 2)])
                        nc.tensor.matmul(
                            ps[:, ga:gb, :], wt[:], apA, start=True, stop=False
                        )
                        apB = _rows_ap(ws, l0, [(2, 2), (3, 2)])
                        nc.tensor.matmul(
                            ps[:, ga:gb, :], wt[:], apB, start=False, stop=False
                        )
                        # image-boundary row: count fix (1.5x conv total); the
                        # clipped 2-row hsum for that row is exactly one q row
                        if ci == 0 and tp == 0 and hp == 0:
                            nc.tensor.matmul(
                                ps[:, 0:1, :],
                                w05[:],
                                qt[:, 0:1, :],
                                start=False,
                                stop=False,
                            )
                        elif ci == NCHUNK - 1 and tp == NTILE - 1 and hp == 1:
                            nc.tensor.matmul(
                                ps[:, TILE_ROWS - 1 : TILE_ROWS, :],
                                w05[:],
                                qt[:, HC // 2 - 1 : HC // 2, :],
                                start=False,
                                stop=False,
                            )
                        # residual +x rides the same PSUM group
                        nc.tensor.matmul(
                            ps[:, ga:gb, :],
                            it[:],
                            xt[:, h0 + l0 : h0 + l0 + GROUP_ROWS, :],
                            start=False,
                            stop=True,
                        )
                    nc.scalar.activation(
                        ot[:, t0 : t0 + TILE_ROWS, :],
                        ps[:],
                        IDENT_FN,
                        bias=bt[:],
                    )
                nc.sync.dma_start(out=yap[p0 : p0 + 128, h0 : h0 + HC, :], in_=ot[:])
    nc.compile()
    return nc


_NC = None


def _get_nc() -> bass.Bass:
    global _NC
    if _NC is None:
        _NC = _build_nc()
    return _NC


def _host_inputs(x: np.ndarray, conv_w: np.ndarray, conv_b: np.ndarray):
    import ml_dtypes

    bf = ml_dtypes.bfloat16
    conv_w = np.asarray(conv_w)
    conv_b = np.asarray(conv_b)
    x = np.asarray(x)
    w9t = np.zeros((2 * C, 2 * C), dtype=np.float32)
    wT = (conv_w.astype(np.float32) / 9.0).T
    w9t[0:C, 0:C] = wT
    w9t[C : 2 * C, C : 2 * C] = wT
    ident = np.eye(2 * C, dtype=np.float32).astype(bf)
    w05t = (w9t * 0.5).astype(bf)
    bias2 = np.concatenate([conv_b, conv_b]).reshape(2 * C, 1).astype(np.float32)
    x = np.ascontiguousarray(x, dtype=np.float32).astype(bf)
    in_maps = []
    for i in range(NCORES):
        xi = x[i * PER : (i + 1) * PER].reshape(PER * C, H, W)
        in_maps.append(
            {
                "x": xi,
                "w9t": w9t.astype(bf),
                "ident": ident,
                "w05t": w05t,
                "bias2": bias2,
            }
        )
    return in_maps


def kernel(x: np.ndarray, conv_w: np.ndarray, conv_b: np.ndarray) -> np.ndarray:
    nc = _get_nc()
    in_maps = _host_inputs(x, conv_w, conv_b)
    res = run_bass_kernel_spmd(nc, in_maps, list(range(NCORES)))
    outs = [
        np.asarray(res.results[i]["y"]).astype(np.float32).reshape(PER, C, H, W)
        for i in range(NCORES)
    ]
    return np.concatenate(outs, axis=0)



# revision 13
# speedup vs baseline: 1.2717x; 1.2717x over previous
"""Trainium2 Bass kernel for nn_Denoising_ResNet: out = x + conv1x1(box_mean3x3(x)) + b.

Sharding: data-parallel over batch (32 samples -> 4 per core x 8 cores).

Per-core layout: 2 "stacks" of 2 samples each -> 128 SBUF partitions
(= 2 samples x 64 channels). Each stack's full image is cast-loaded to
one bf16 SBUF tile by 4 quarter DMAs whose boundaries (34/66/98) align
with the 32-row compute chunks' halo spans, so chunk ci only waits on
quarters 0..ci.

Math decomposition (K=3 edge-clipped box mean, then 1x1 conv):
  - W-direction 3-tap sum: two shifted DVE adds (bf16, 2x DVE mode);
    edge cols folded to 1.5*(2-tap).
  - Global /9 of the box mean is folded into the conv weight.
  - H-direction 3-tap sum is FUSED into the 1x1 conv: 3 accumulating
    PE matmuls with row-shifted moving operands against a block-diagonal
    [128,128] weight kron(I2, (W/9)^T) in bf16; zeroed ws halo rows give
    the edge-clipped sum at image boundaries.
  - The residual +x rides the same PSUM group as a 4th accumulating
    identity matmul (kron(I2, I) bf16).
  - Image-boundary rows get 2 extra in-group matmuls against 0.5x the
    conv weight (-> 1.5x conv total = the edge-clip row count fix,
    leaving x and bias unscaled).
  - One accumulation group per 2KB PSUM bank (start=True zeroes the
    whole bank); matmuls are emitted grouped by stationary weight.
  - PSUM -> SBUF copy + bias live on the scalar engine
    (activation Identity, bias per partition), one per 2-bank PSUM tile.

Measured: ~95.2us HW exec vs ~94us HBM roofline (16.8MB read +
16.8MB write per core at ~358 GB/s/core).
"""
from contextlib import ExitStack

import bass_rust
import numpy as np

import concourse.bass as bass
import concourse.tile as tile
from concourse import bacc, mybir
from concourse.bass_utils import run_bass_kernel_spmd

B, C, H, W = 32, 64, 128, 128
NCORES = 8
PER = B // NCORES  # samples per core
NSTACK = PER // 2  # 2-sample stacks per core
HC = 32  # chunk height (output rows per chunk)
NCHUNK = H // HC
GROUP_ROWS = 4  # rows per matmul accumulation group (512 f32 = 1 bank)
TILE_ROWS = 16  # rows per PSUM tile (4 banks), 4 groups per tile
NTILE = HC // TILE_ROWS
NGRP = TILE_ROWS // GROUP_ROWS

F32 = mybir.dt.float32
BF16 = mybir.dt.bfloat16


def _rows_ap(t, base_row, dims):
    """AP over a [128, R, W] tile starting at base_row, with custom row dims.

    dims: list of (row_stride_in_rows, size) free dims; an inner (1, W) dim is
    appended. Strides may be 0 (repeat) or overlap — used to stream repeated /
    permuted row patterns into the PE without materializing them in SBUF.
    """
    ap = t[:, base_row : base_row + 1, :].copy()
    part = ap.ap.to_list()[0]
    ap.ap = bass_rust.VecI64Pair([part] + [[s * W, n] for s, n in dims] + [[1, W]])
    return ap


def _build_nc() -> bass.Bass:
    nc = bacc.Bacc("TRN2", debug=False)
    x = nc.dram_tensor("x", [PER * C, H, W], BF16, kind="ExternalInput")
    w9t = nc.dram_tensor("w9t", [2 * C, 2 * C], BF16, kind="ExternalInput")
    ident = nc.dram_tensor("ident", [2 * C, 2 * C], BF16, kind="ExternalInput")
    w05t = nc.dram_tensor("w05t", [2 * C, 2 * C], BF16, kind="ExternalInput")
    bias2 = nc.dram_tensor("bias2", [2 * C, 1], F32, kind="ExternalInput")
    y = nc.dram_tensor("y", [PER * C, H, W], BF16, kind="ExternalOutput")
    xap = x.ap()
    yap = y.ap()
    IDENT_FN = mybir.ActivationFunctionType.Identity

    with ExitStack() as ctx:
        tc = ctx.enter_context(tile.TileContext(nc))
        cpool = ctx.enter_context(tc.tile_pool(name="const", bufs=1))
        wt = cpool.tile([128, 128], BF16)
        nc.sync.dma_start(out=wt[:], in_=w9t.ap()[:, :])
        it = cpool.tile([128, 128], BF16)
        nc.sync.dma_start(out=it[:], in_=ident.ap()[:, :])
        w05 = cpool.tile([128, 128], BF16)
        nc.sync.dma_start(out=w05[:], in_=w05t.ap()[:, :])
        bt = cpool.tile([128, 1], F32)
        nc.sync.dma_start(out=bt[:], in_=bias2.ap()[:, :])

        ppool = ctx.enter_context(tc.tile_pool(name="psum", bufs=2, space="PSUM"))

        xpool = ctx.enter_context(tc.tile_pool(name="xin", bufs=2))
        tpool = ctx.enter_context(tc.tile_pool(name="tmp", bufs=2))
        wpool = ctx.enter_context(tc.tile_pool(name="wsum", bufs=2))
        qpool = ctx.enter_context(tc.tile_pool(name="qsum", bufs=2))
        opool = ctx.enter_context(tc.tile_pool(name="out", bufs=4))

        for g in range(NSTACK):
            p0 = g * 128
            # whole-stack bf16 image tile, filled by 4 quarter cast-DMAs
            # (no halo re-reads; chunks slice it with their halos).
            xt = xpool.tile([128, H, W], BF16)
            # quarter boundaries at 34/66/98: chunk ci's halo span
            # [32*ci-1, 32*ci+33) is covered by quarters 0..ci, so each
            # chunk waits only one new quarter-DMA.
            qb = [0, HC + 2, 2 * HC + 2, 3 * HC + 2, H]
            for q in range(4):
                nc.gpsimd.dma_start(
                    out=xt[:, qb[q] : qb[q + 1], :],
                    in_=xap[p0 : p0 + 128, qb[q] : qb[q + 1], :],
                )
            for ci in range(NCHUNK):
                h0 = ci * HC
                # chunk rows [h0, h0+HC); halo rows clamped at the image edge
                ra = 0 if ci == 0 else h0 - 1       # first xt row read
                rb = h0 + HC if ci == NCHUNK - 1 else h0 + HC + 1

                # W-direction 3-tap on DVE (bf16). tt/ws row r holds
                # image row h0-1+r; out-of-image halo ws rows are zeroed.
                la = ra - (h0 - 1)  # first valid local row (0 or 1)
                lb = rb - (h0 - 1)  # past-last valid local row
                tt = tpool.tile([128, HC + 2, W], BF16)
                ws = wpool.tile([128, HC + 2, W], BF16)
                if ci == 0:
                    nc.vector.memset(ws[:, 0:1, :], 0.0)
                elif ci == NCHUNK - 1:
                    nc.vector.memset(ws[:, HC + 1 : HC + 2, :], 0.0)
                nc.vector.tensor_add(
                    tt[:, la:lb, 1:W], xt[:, ra:rb, 0 : W - 1], xt[:, ra:rb, 1:W]
                )
                nc.vector.tensor_add(
                    ws[:, la:lb, 1 : W - 1], tt[:, la:lb, 1 : W - 1], xt[:, ra:rb, 2:W]
                )
                nc.vector.tensor_scalar_mul(ws[:, la:lb, 0:1], tt[:, la:lb, 1:2], 1.5)
                nc.vector.tensor_scalar_mul(
                    ws[:, la:lb, W - 1 : W], tt[:, la:lb, W - 1 : W], 1.5
                )

                # even-aligned H pair sums on DVE:
                # q[j] = ws_img[h0+2j] + ws_img[h0+2j+1]  (locals 2j+1, 2j+2)
                qt = qpool.tile([128, HC // 2, W], BF16)
                nc.vector.tensor_add(
                    qt[:], ws[:, 1 : HC + 1 : 2, :], ws[:, 2 : HC + 2 : 2, :]
                )

                ot = opool.tile([128, HC, W], BF16)
                pss = []
                for _pt in range(NTILE):
                    ps_t = ppool.tile([128, TILE_ROWS, W], F32, tag="ps", name="ps_t")
                    pss.append(ps_t)
                # residual pass C first (start=True): only needs xt, so the PE
                # starts as soon as the quarter-DMA lands — the DVE ws/q chain
                # is off the PE's critical path at chunk start.
                for tp in range(NTILE):
                    t0 = tp * TILE_ROWS
                    for hp in range(NGRP):
                        ga, gb = hp * GROUP_ROWS, (hp + 1) * GROUP_ROWS
                        nc.tensor.matmul(
                            pss[tp][:, ga:gb, :],
                            it[:],
                            xt[:, h0 + t0 + ga : h0 + t0 + gb, :],
                            start=True,
                            stop=False,
                        )
                # H-direction 3-tap fused into the conv with only 2 PE
                # passes per 4-row group via the pair sums:
                #   even r: hsum[r] = q[r/2]     + ws[r-1]
                #   odd  r: hsum[r] = q[(r-1)/2] + ws[r+1]
                # pass B streams ws locals l0+(0,3,2,5) [overlapping dims],
                # pass A streams q rows (j0,j0,j0+1,j0+1) [stride-0 repeat].
                for tp in range(NTILE):
                    ps = pss[tp]
                    t0 = tp * TILE_ROWS
                    for hp in range(NGRP):
                        ga, gb = hp * GROUP_ROWS, (hp + 1) * GROUP_ROWS
                        l0 = t0 + ga  # chunk-local first output row of group
                        apB = _rows_ap(ws, l0, [(2, 2), (3, 2)])
                        nc.tensor.matmul(
                            ps[:, ga:gb, :], wt[:], apB, start=False, stop=False
                        )
                        apA = _rows_ap(qt, l0 // 2, [(1, 2), (0, 2)])
                        # image-boundary row: count fix (1.5x conv total); the
                        # clipped 2-row hsum for that row is exactly one q row
                        fix = None
                        if ci == 0 and l0 == 0:
                            fix = 0
                        elif ci == NCHUNK - 1 and l0 == HC - GROUP_ROWS:
                            fix = HC // 2 - 1
                        nc.tensor.matmul(
                            ps[:, ga:gb, :], wt[:], apA, start=False, stop=(fix is None)
                        )
                        if fix is not None:
                            fr = 0 if fix == 0 else TILE_ROWS - 1
                            nc.tensor.matmul(
                                ps[:, fr : fr + 1, :],
                                w05[:],
                                qt[:, fix : fix + 1, :],
                                start=False,
                                stop=True,
                            )
                    nc.scalar.activation(
                        ot[:, t0 : t0 + TILE_ROWS, :],
                        ps[:],
                        IDENT_FN,
                        bias=bt[:],
                    )
                nc.sync.dma_start(out=yap[p0 : p0 + 128, h0 : h0 + HC, :], in_=ot[:])
    nc.compile()
    return nc


_NC = None


def _get_nc() -> bass.Bass:
    global _NC
    if _NC is None:
        _NC = _build_nc()
    return _NC


def _host_inputs(x: np.ndarray, conv_w: np.ndarray, conv_b: np.ndarray):
    import ml_dtypes

    bf = ml_dtypes.bfloat16
    conv_w = np.asarray(conv_w)
    conv_b = np.asarray(conv_b)
    x = np.asarray(x)
    w9t = np.zeros((2 * C, 2 * C), dtype=np.float32)
    wT = (conv_w.astype(np.float32) / 9.0).T
    w9t[0:C, 0:C] = wT
    w9t[C : 2 * C, C : 2 * C] = wT
    ident = np.eye(2 * C, dtype=np.float32).astype(bf)
    w05t = (w9t * 0.5).astype(bf)
    bias2 = np.concatenate([conv_b, conv_b]).reshape(2 * C, 1).astype(np.float32)
    x = np.ascontiguousarray(x, dtype=np.float32).astype(bf)
    in_maps = []
    for i in range(NCORES):
        xi = x[i * PER : (i + 1) * PER].reshape(PER * C, H, W)
        in_maps.append(
            {
                "x": xi,
                "w9t": w9t.astype(bf),
                "ident": ident,
                "w05t": w05t,
                "bias2": bias2,
            }
        )
    return in_maps


def kernel(x: np.ndarray, conv_w: np.ndarray, conv_b: np.ndarray) -> np.ndarray:
    nc = _get_nc()
    in_maps = _host_inputs(x, conv_w, conv_b)
    res = run_bass_kernel_spmd(nc, in_maps, list(range(NCORES)))
    outs = [
        np.asarray(res.results[i]["y"]).astype(np.float32).reshape(PER, C, H, W)
        for i in range(NCORES)
    ]
    return np.concatenate(outs, axis=0)

